# revision 43
# baseline (speedup 1.0000x reference)
"""Trainium2 Bass kernel for nn_CA1AttentionGate.

Computes, for full inputs (B=1, S=8192, H=1024, F=128, K=2):
    temporal = relu(t @ Wt1 + bt1) @ Wt2 + bt2          [K,F]
    mem      = dg_features + temporal                    [K,F]
    qmean    = query.mean(axis=1)                        [1,H]
    score_k  = tanh([mem_k ; qmean] @ Wa1 + ba1) @ Wa2 + ba2
    w_k      = sigmoid(score_k)
    g_k      = mem_k @ Wg + bg                           [K,H]
    row[s]   = (1/K) * sum_k w_k * (g_k . key[s])        [S]
    out      = broadcast(row) -> [1,1,S,S]

Sharding: sequence-parallel over the key/seq axis across 8 cores.  Each
core computes the final gate row for its 1024 key positions and writes
its dense [8192, 1024] column slab of the output (the 93us DMA pole).
The only cross-core quantity is qmean: a 4KB AllGather.

Schedule highlights (per core, cost-model timings):
  - query ships from the host in bf16 (it only feeds the seq-mean, which
    accumulates in f32 PSUM; ~4e-5 of the 2e-2 error budget): the read
    that gates the AllGather halves to 2MB, and with the cc_in store
    preempting the key stream the collective fires ~14.5us (vs 27us
    naive).  bf16 KEYS do not work — that variant returns garbage on
    real HW (strided bf16 DMA or gpsimd bf16 mul, unbisected).  The
    g-path matmuls run f32r (1 cycle/row vs 4 for fp32, ~2e-4 rel err).
  - weights ride in 3 packed DMAs (one [128,1290] f32 block, Wg in f32r,
    one [1,1153] row) instead of 13 singles, pulling key tiles ~4us
    earlier.
  - key matvec muls split DVE (12) / Pool (4); reductions split ACT
    accum-copies (13) / DVE tensor_reduce (3) into disjoint accumulator
    tiles — a shared tile would serialize the engines through per-tile
    WAW deps, and tensor_tensor_reduce (the fused alternative) crashes
    the NRT exec unit on real hardware.
  - DMA triggers for post-collective loads sit on the otherwise-idle SP
    ring: a trigger occupies the issuing engine's sequencer while it
    waits, so parking it on ACT would stall the accum stream behind the
    collective.
  - the post-collective scorer runs on PE rank-1 accumulations straight
    off the gathered buffer (no DVE reduce on the critical path); the
    sigmoid weights land ~37us and output writes start ~42us of the
    ~137us total (the [8192,1024] slab write is a fixed ~93us pole).
  Model 136,830ns vs 155,339 baseline; HW rel err 2.1e-4.
"""

import os

import numpy as np

SEQ = 8192
H = 1024
F = 128
K = 2
NCORES = 8
SHARD = SEQ // NCORES  # 1024
NT = SHARD // 128  # 8 key tiles per shard

# packed-f32 weight block column offsets ([128, WPACK_COLS])
_C_TB = 0        # [32, 2]   ts broadcast
_C_WT1 = 2       # [32, 1]
_C_BT1 = 3       # [32, 1]
_C_WT2 = 4       # [32, 128]
_C_BT2 = 132     # [128, 1]
_C_DG = 133      # [128, 2]  dg_features^T
_C_WA1M = 135    # [128, 128]
_C_WA1Q = 263    # [128, 8*128] qmean rows of Wa1, interleaved-qmT pairing
_C_WA2 = 1287    # [128, 1]
_C_BA1 = 1288    # [128, 1]
_C_ONE = 1289    # [128, 1]  constant 1.0
WPACK_COLS = 1290

_PROG_CACHE = {}


def _build(use_collective: bool):
    import concourse.bacc as bacc
    import concourse.bass as bass
    import concourse.tile as tile
    from concourse import mybir
    from concourse.tile_rust import add_dep_helper

    AF = mybir.ActivationFunctionType
    ALU = mybir.AluOpType
    f32 = mybir.dt.float32
    # f32r: PE runs fp32 data single-pass (1 cycle/row vs 4 for plain fp32)
    # at slightly reduced internal precision — far inside the 2e-2 gate.
    f32r = mybir.dt.float32r
    bf16 = mybir.dt.bfloat16

    nc = bacc.Bacc(
        "TRN2",
        target_bir_lowering=False,
        debug=False,
        num_devices=NCORES,
    )

    def din(name, shape, dt=None):
        return nc.dram_tensor(
            name, list(shape), dt or f32, kind="ExternalInput"
        ).ap()

    q_rows = SHARD if use_collective else SEQ
    qs = din("qs", (q_rows, H), bf16)
    ks = din("ks", (SHARD, H))
    wpack = din("wpack", (128, WPACK_COLS))
    wg = din("wg", (F, H), f32r)
    vpack = din("vpack", (1, H + 1 + F))  # bg ++ ba2 ++ ba1_row
    scale_col = din("scale_col", (128, 1), bf16)
    out = nc.dram_tensor("out", [SEQ, SHARD], f32, kind="ExternalOutput").ap()

    with tile.TileContext(nc) as tc:
        with (
            tc.tile_pool(name="consts", bufs=1) as cp,
            tc.tile_pool(name="work", bufs=1) as wp,
            tc.tile_pool(name="qstream", bufs=8) as qp,
            tc.tile_pool(name="scratch", bufs=4) as sp,
            tc.tile_pool(name="scratch_p", bufs=4) as spp,
            tc.tile_pool(name="scratch_a", bufs=2) as spa,
            tc.tile_pool(name="psum_small", bufs=2, space="PSUM") as pps,
            tc.tile_pool(name="psum_big", bufs=3, space="PSUM") as ppb,
            tc.tile_pool(name="dram", bufs=1, space="DRAM") as dp,
        ):
            # ---- query shard DMAs own the wire from t=0 ----------------
            nq = q_rows // 128
            qv = qs.rearrange("(t p) h -> t p h", p=128)
            qtiles = []
            for i in range(nq):
                qt = qp.tile([128, H], bf16, tag="qt")
                nc.sync.dma_start(qt, qv[i])
                qtiles.append(qt)

            # scale column rides the (otherwise idle) ACT ring so the PE
            # reduce can start as soon as the first query tile lands
            sc_c = cp.tile([128, 1], bf16)
            nc.scalar.dma_start(sc_c, scale_col)

            # ---- weights: 3 packed DMAs on the sync ring ---------------
            wpk = cp.tile([128, WPACK_COLS], f32)
            nc.sync.dma_start(wpk, wpack)
            Wg_sb = cp.tile([F, H], f32r)
            nc.sync.dma_start(Wg_sb, wg)
            vpk = cp.tile([1, H + 1 + F], f32)
            nc.sync.dma_start(vpk, vpack)

            tb_sb = wpk[0:32, _C_TB : _C_TB + 2]
            Wt1T_sb = wpk[0:32, _C_WT1 : _C_WT1 + 1]
            bt1T_sb = wpk[0:32, _C_BT1 : _C_BT1 + 1]
            Wt2_sb = wpk[0:32, _C_WT2 : _C_WT2 + 128]
            bt2T_sb = wpk[:, _C_BT2 : _C_BT2 + 1]
            dgT_sb = wpk[:, _C_DG : _C_DG + 2]
            Wa1m_sb = wpk[:, _C_WA1M : _C_WA1M + 128]
            Wa1q_sb = wpk[:, _C_WA1Q : _C_WA1Q + 1024].rearrange(
                "p (c f) -> p c f", c=8
            )
            Wa2_sb = wpk[:, _C_WA2 : _C_WA2 + 1]
            ba1T_sb = wpk[:, _C_BA1 : _C_BA1 + 1]
            one11 = wpk[0:1, _C_ONE : _C_ONE + 1]
            bg_sb = vpk[:, 0:H]
            ba2b_sb = vpk[:, H : H + 1]
            ba1row = vpk[:, H + 1 : H + 1 + F]

            # warm the ACT function tables used late in the critical path
            warm1 = cp.tile([1, 1], f32)
            nc.scalar.activation(warm1, sc_c[0:1, :], AF.Tanh)
            warm2 = cp.tile([1, 1], f32)
            nc.scalar.activation(warm2, sc_c[0:1, :], AF.Sigmoid)

            # key shard: interleaved, ktiles[j][p, :] = ks[p*NT + j, :]
            kv = ks.rearrange("(p t) h -> p t h", t=NT)
            ktiles = []
            k_insts = []
            for j in range(NT):
                kt = cp.tile([128, H], f32, tag=f"ks{j}")
                k_insts.append(nc.sync.dma_start(kt, kv[:, j, :]))
                ktiles.append(kt)

            # ---- qsum on the PE as each tile arrives (f32r) ------------
            qsum_ps = ppb.tile([1, H], f32, tag="big")
            for i in range(nq):
                nc.tensor.matmul(
                    qsum_ps[:, 0:512], lhsT=sc_c, rhs=qtiles[i][:, 0:512],
                    start=(i == 0), stop=(i == nq - 1),
                )
                nc.tensor.matmul(
                    qsum_ps[:, 512:1024], lhsT=sc_c, rhs=qtiles[i][:, 512:1024],
                    start=(i == 0), stop=(i == nq - 1),
                )

            qpart_sb = wp.tile([1, H], f32)
            nc.scalar.copy(qpart_sb, qsum_ps)
            if use_collective:
                cc_in = dp.tile([1, H], f32)
                cc_out = dp.tile([NCORES, H], f32)
                cci = nc.scalar.dma_start(cc_in, qpart_sb)
                for ki in k_insts:
                    add_dep_helper(ki.ins, cci.ins,
                                   reason="key reads yield to cc_in")

            # ---- temporal MLP -> memT [F, K] ---------------------------
            h1T = wp.tile([F // 4, K], f32)
            nc.vector.tensor_scalar_mul(h1T, tb_sb, Wt1T_sb)
            nc.vector.tensor_scalar_add(h1T, h1T, bt1T_sb)
            nc.vector.tensor_relu(h1T, h1T)
            tT_ps = pps.tile([F, K], f32, tag="small")
            nc.tensor.matmul(tT_ps, lhsT=Wt2_sb, rhs=h1T, start=True, stop=True)
            memT_sb = wp.tile([F, K], f32)
            nc.scalar.activation(memT_sb, tT_ps, AF.Identity, bias=bt2T_sb, scale=1.0)
            nc.vector.tensor_add(memT_sb, memT_sb, dgT_sb)
            memTr = wp.tile([F, K], f32r)
            nc.vector.tensor_copy(memTr, memT_sb)

            # ---- gate rows g_k = mem_k @ Wg + bg  [1, H] (f32r) --------
            def g_row(k):
                g_ps = ppb.tile([1, H], f32, tag="big")
                nc.tensor.matmul(g_ps[:, 0:512], lhsT=memTr[:, k : k + 1],
                                 rhs=Wg_sb[:, 0:512], start=True, stop=True)
                nc.tensor.matmul(g_ps[:, 512:1024], lhsT=memTr[:, k : k + 1],
                                 rhs=Wg_sb[:, 512:1024], start=True, stop=True)
                return g_ps

            g0_ps = g_row(0)
            g1_ps = g_row(1)
            g0_sb = wp.tile([1, H], f32, tag="g0r")
            nc.vector.tensor_add(g0_sb, g0_ps, bg_sb)
            g1_sb = wp.tile([1, H], f32, tag="g1r")
            nc.vector.tensor_add(g1_sb, g1_ps, bg_sb)

            # Pool order: gb0 first (unblocks DVE), collective, gb1, muls
            gb0 = wp.tile([128, H], f32, tag="gb0")
            nc.gpsimd.partition_broadcast(gb0[:, :], g0_sb[:, :])
            if use_collective:
                nc.gpsimd.collective_compute(
                    "AllGather",
                    ALU.bypass,
                    replica_groups=[list(range(NCORES))],
                    ins=[cc_in.opt()],
                    outs=[cc_out.opt()],
                )
                # gathered partials, interleaved-reshape layout, ACT ring
                qmTd8 = wp.tile([128, NCORES, 8], f32)
                nc.sync.dma_start(
                    qmTd8, cc_out[:, :].rearrange("d (p c) -> p d c", c=8)
                )
            gb1 = wp.tile([128, H], f32, tag="gb1")
            nc.gpsimd.partition_broadcast(gb1[:, :], g1_sb[:, :])

            # ---- matvec: row_k[p*NT+j] = sum_h g_k[h] * ks[p*NT+j, h] --
            # Muls on DVE (10) + Pool (6); reductions on ACT (12, into rcc)
            # and DVE (4, into rccD).  Writers stay disjoint per tile so the
            # engines don't serialize through per-tile WAW deps.
            # (tensor_tensor_reduce would fuse mul+reduce on DVE but crashes
            # the NRT exec unit on real hardware.)
            rcc = wp.tile([128, NT * K], f32)   # ACT: cols 2j, and 2j+1 j<4
            rccD = wp.tile([128, 3], f32)       # DVE: anchor1 j>=5

            def act_accum(prod, acc):
                junk = spa.tile([128, H], f32, tag="ajunk")
                nc.scalar.activation(junk, prod, AF.Copy, accum_out=acc)

            def dve_mul(gb, j):
                prod = sp.tile([128, H], f32, tag="dprod")
                nc.vector.tensor_mul(prod, ktiles[j], gb)
                return prod

            def pool_mul(gb, j):
                prod = spp.tile([128, H], f32, tag="prod")
                nc.gpsimd.tensor_mul(prod, ktiles[j], gb)
                return prod

            pool_prods = {}
            for j in range(4):
                act_accum(dve_mul(gb0, j), rcc[:, 2 * j : 2 * j + 1])
                act_accum(dve_mul(gb1, j), rcc[:, 2 * j + 1 : 2 * j + 2])
            def dve_reduce(j):
                nc.vector.tensor_reduce(
                    rccD[:, j - 5 : j - 4],
                    pool_prods[j][:, :].rearrange("p (a h) -> p a h", a=1),
                    axis=mybir.AxisListType.X, op=ALU.add,
                )

            # reduces interleave between the remaining k0 muls so DVE is
            # drained (and free for the scorer) right after the last prod
            act_accum(dve_mul(gb0, 4), rcc[:, 8:9])
            act_accum(pool_mul(gb1, 4), rcc[:, 9:10])
            for j in range(5, NT):
                act_accum(dve_mul(gb0, j), rcc[:, 2 * j : 2 * j + 1])
                pool_prods[j] = pool_mul(gb1, j)
                if j - 1 in pool_prods:
                    dve_reduce(j - 1)
            dve_reduce(NT - 1)

            # ---- post-collective scorer --------------------------------
            # mem part of the pre-activation [F, K] (ready pre-collective)
            haTm_ps = pps.tile([F, K], f32, tag="small")
            nc.tensor.matmul(haTm_ps, lhsT=Wa1m_sb, rhs=memT_sb,
                             start=True, stop=True)
            # qmean part: accumulate the gathered per-core partials directly
            # on the PE (65 rank-1 matmuls at ~7ns each beat a DVE reduce
            # that would queue behind the matvec) + ba1 via ba1row^T @ [1.0]
            haTq_ps = pps.tile([F, 1], f32, tag="small")
            if use_collective:
                for c in range(8):
                    for d in range(NCORES):
                        nc.tensor.matmul(haTq_ps, lhsT=Wa1q_sb[:, c, :],
                                         rhs=qmTd8[:, d, c : c + 1],
                                         start=(c == 0 and d == 0), stop=False)
            else:
                qmT = wp.tile([128, 8], f32)
                nc.scalar.dma_start(qmT, qpart_sb[:, :])
                for c in range(8):
                    nc.tensor.matmul(haTq_ps, lhsT=Wa1q_sb[:, c, :],
                                     rhs=qmT[:, c : c + 1],
                                     start=(c == 0), stop=False)
            nc.tensor.matmul(haTq_ps, lhsT=ba1row, rhs=one11,
                             start=False, stop=True)
            # bias port carries (qmean part + ba1), shared by both anchors
            hq_sb = wp.tile([F, 1], f32)
            nc.scalar.copy(hq_sb, haTq_ps)
            aT_sb = wp.tile([F, K], f32)
            nc.scalar.activation(aT_sb, haTm_ps, AF.Tanh, bias=hq_sb, scale=1.0)
            scoreT_ps = pps.tile([1, K], f32, tag="small")
            nc.tensor.matmul(scoreT_ps, lhsT=Wa2_sb, rhs=aT_sb, start=True, stop=True)
            wvT_sb = wp.tile([1, K], f32)
            nc.scalar.activation(wvT_sb, scoreT_ps, AF.Sigmoid, bias=ba2b_sb, scale=1.0)
            wvb = wp.tile([128, K], f32, tag="wvb")
            nc.gpsimd.partition_broadcast(wvb[:, :], wvT_sb[:, :])

            # ---- combine anchors: o128[p, j] = (w0 r0 + w1 r1) / K -----
            rc = rcc[:, :]
            r_ev = bass.AP(tensor=rc.tensor, offset=rc.offset,
                           ap=[list(rc.ap[0]), [2, NT]])
            r_od5 = bass.AP(tensor=rc.tensor, offset=rc.offset + 1,
                            ap=[list(rc.ap[0]), [2, 5]])
            o128 = wp.tile([128, NT], f32)
            o128b = wp.tile([128, NT], f32)
            nc.vector.tensor_scalar(o128, r_ev, wvb[:, 0:1], 1.0 / K,
                                    op0=ALU.mult, op1=ALU.mult)
            nc.vector.tensor_scalar(o128b[:, 0:5], r_od5, wvb[:, 1:2], 1.0 / K,
                                    op0=ALU.mult, op1=ALU.mult)
            nc.vector.tensor_scalar(o128b[:, 5:8], rccD, wvb[:, 1:2], 1.0 / K,
                                    op0=ALU.mult, op1=ALU.mult)
            nc.vector.tensor_add(o128, o128, o128b)

            # flatten (p, j) -> s on the ACT ring, broadcast, write out
            o_row = wp.tile([1, SHARD], f32)
            nc.sync.dma_start(o_row[:, :], o128[:, :])
            out_sb = wp.tile([128, SHARD], f32)
            nc.gpsimd.partition_broadcast(out_sb[:, :], o_row[:, :])

            # ---- output: 64 x [128 rows, SHARD cols], all rows = row ---
            outv = out.rearrange("(b p) n -> b p n", p=128)
            for b in range(SEQ // 128):
                nc.sync.dma_start(outv[b], out_sb)

    nc.compile()
    return nc


def _get_prog(use_collective: bool):
    key = bool(use_collective)
    if key not in _PROG_CACHE:
        _PROG_CACHE[key] = _build(key)
    return _PROG_CACHE[key]


def _pack_weights(inputs):
    a = lambda n: np.asarray(inputs[n], np.float32)
    wpk = np.zeros((128, WPACK_COLS), np.float32)
    wpk[0:32, _C_TB : _C_TB + 2] = np.broadcast_to(a("timestamps"), (32, K))
    wpk[0:32, _C_WT1] = a("Wt1").reshape(-1)
    wpk[0:32, _C_BT1] = a("bt1")
    wpk[0:32, _C_WT2 : _C_WT2 + 128] = a("Wt2")
    wpk[:, _C_BT2] = a("bt2")
    wpk[:, _C_DG : _C_DG + 2] = a("dg_features").T
    wpk[:, _C_WA1M : _C_WA1M + 128] = a("Wa1")[0:F]
    # Wa1q_sb[p, c, f] = Wa1[F + p*8 + c, f]
    wpk[:, _C_WA1Q : _C_WA1Q + 1024] = a("Wa1")[F : F + H].reshape(128, 1024)
    wpk[:, _C_WA2] = a("Wa2").reshape(-1)
    wpk[:, _C_BA1] = a("ba1")
    wpk[:, _C_ONE] = 1.0
    vpk = np.empty((1, H + 1 + F), np.float32)
    vpk[0, 0:H] = a("bg")
    vpk[0, H] = a("ba2").reshape(-1)[0]
    vpk[0, H + 1 :] = a("ba1")
    return np.ascontiguousarray(wpk), np.ascontiguousarray(vpk)


def _make_in_maps(inputs, use_collective: bool):
    import ml_dtypes
    # query rides in bf16: it only feeds the seq-mean (f32 PSUM accum), so
    # rounding costs ~4e-5 output rel err against the 2e-2 gate, and the
    # 4MB/core read that gates the AllGather launch halves to 2MB.
    q = np.ascontiguousarray(
        np.asarray(inputs["query"], np.float32)[0].astype(ml_dtypes.bfloat16)
    )  # [S,H] bf16
    k = np.ascontiguousarray(np.asarray(inputs["key"], np.float32)[0])  # [S,H]
    wpk, vpk = _pack_weights(inputs)
    common = {
        "wpack": wpk,
        "wg": np.ascontiguousarray(np.asarray(inputs["Wg"], np.float32)),
        "vpack": vpk,
        "scale_col": np.full((128, 1), 1.0 / 8192.0, np.float32).astype(
            ml_dtypes.bfloat16
        ),
    }
    in_maps = []
    for d in range(NCORES):
        m = dict(common)
        m["ks"] = np.ascontiguousarray(k[d * SHARD : (d + 1) * SHARD])
        if use_collective:
            m["qs"] = np.ascontiguousarray(q[d * SHARD : (d + 1) * SHARD])
        else:
            m["qs"] = q
        in_maps.append(m)
    return in_maps


def _run(inputs, use_collective: bool, trace: bool = False):
    from concourse.bass_utils import run_bass_kernel_spmd

    nc = _get_prog(use_collective)
    in_maps = _make_in_maps(inputs, use_collective)
    res = run_bass_kernel_spmd(
        nc, in_maps, core_ids=list(range(NCORES)), trace=trace
    )
    full = np.empty((1, 1, SEQ, SEQ), np.float32)
    for d in range(NCORES):
        full[0, 0, :, d * SHARD : (d + 1) * SHARD] = res.results[d]["out"]
    return full, res


def kernel(**inputs) -> np.ndarray:
    use_collective = os.environ.get("CA1_NO_COLLECTIVE", "0") != "1"
    try:
        full, _ = _run(inputs, use_collective)
        return full
    except Exception:
        if not use_collective:
            raise
        # fall back to the zero-communication variant (replicated query)
        _PROG_CACHE.pop(True, None)
        full, _ = _run(inputs, False)
        return full
